# revision 1
# baseline (speedup 1.0000x reference)
"""Single-read variant: one natural DMA per chunk; xT derived on-chip via
PE transposes. Staggered emission so each engine's in-order stream only
meets work whose inputs are >=1 iteration old:
  PE:  tp(t) | h(t-1) | s(t-2) | o(t-3)
  DVE: S_w(t-3) | copies(t)
  ACT: tanh(t-1) | exp(t-2)
"""
import sys

if "/opt/trn_rl_repo" not in sys.path:
    sys.path.insert(0, "/opt/trn_rl_repo")

import ml_dtypes
import numpy as np

import concourse.bacc as bacc
import concourse.tile as tile
from concourse import bass_utils, mybir
from concourse.alu_op_type import AluOpType

C = 8
G = 1024
SPC = G // C
D = 256
H = 128
CHUNK = 1024
TPC = CHUNK // 128
NSLOT = 8

F32 = mybir.dt.float32
BF16 = mybir.dt.bfloat16

_cache: dict = {}


def _build(npad: int):
    nchunks = npad // CHUNK
    ntiles = npad // 128
    nc = bacc.Bacc("TRN2", target_bir_lowering=False, debug=False, num_devices=C)

    x_d = nc.dram_tensor("x", [npad, D], BF16, kind="ExternalInput")
    bloc_d = nc.dram_tensor("bloc", [128, ntiles], F32, kind="ExternalInput")
    w1a_d = nc.dram_tensor("w1a", [128, H], BF16, kind="ExternalInput")
    w1b_d = nc.dram_tensor("w1b", [128, H], BF16, kind="ExternalInput")
    w2_d = nc.dram_tensor("w2", [H, 1], BF16, kind="ExternalInput")
    b1_d = nc.dram_tensor("b1", [H, 1], F32, kind="ExternalInput")
    ident_d = nc.dram_tensor("ident", [128, 128], BF16, kind="ExternalInput")
    iota_d = nc.dram_tensor("iota", [128, SPC], F32, kind="ExternalInput")
    cnt_d = nc.dram_tensor("cnt", [SPC, 1], F32, kind="ExternalInput")
    o_d = nc.dram_tensor("o", [SPC, D], F32, kind="ExternalOutput")

    TANH = mybir.ActivationFunctionType.Tanh
    EXP = mybir.ActivationFunctionType.Exp

    with tile.TileContext(nc) as tc:
        with (
            tc.tile_pool(name="const", bufs=1) as constp,
            tc.tile_pool(name="xT", bufs=4) as xTp,
            tc.tile_pool(name="th", bufs=5) as thp,
            tc.tile_pool(name="eb", bufs=8) as ebp,
            tc.tile_pool(name="sw", bufs=24) as swp,
            tc.tile_pool(name="fin", bufs=1) as finp,
            tc.tile_pool(name="tp", bufs=2, space="PSUM") as tpp,
            tc.tile_pool(name="ph", bufs=2, space="PSUM") as php,
            tc.tile_pool(name="ps", bufs=1, space="PSUM") as psp,
            tc.tile_pool(name="po", bufs=1, space="PSUM") as pop,
        ):
            # first two natural reads go ahead of the constant loads so the
            # PE transpose pipeline starts as early as possible
            slots = []
            for s in range(NSLOT):
                sl = constp.tile([128, TPC, D + 1], BF16, tag=f"slot{s}")
                slots.append(sl)
            for t0_, eng in ((0, nc.sync), (1, nc.scalar)):
                eng.dma_start(
                    slots[t0_][:, :, 0:D],
                    x_d[t0_ * CHUNK : (t0_ + 1) * CHUNK, :].rearrange(
                        "(a p) d -> p a d", p=128
                    ),
                )
            for sl in slots:
                nc.gpsimd.memset(sl[:, :, D : D + 1], 1.0)

            w1a = constp.tile([128, H], BF16)
            nc.sync.dma_start(w1a[:], w1a_d[:])
            w1b = constp.tile([128, H], BF16)
            nc.sync.dma_start(w1b[:], w1b_d[:])
            w2 = constp.tile([H, 1], BF16)
            nc.sync.dma_start(w2[:], w2_d[:])
            b1 = constp.tile([H, 1], F32)
            nc.sync.dma_start(b1[:], b1_d[:])
            ident = constp.tile([128, 128], BF16)
            nc.sync.dma_start(ident[:], ident_d[:])
            iota = constp.tile([128, SPC], F32)
            nc.sync.dma_start(iota[:], iota_d[:])
            cnt = constp.tile([SPC, 1], F32)
            nc.sync.dma_start(cnt[:], cnt_d[:])
            bloc = constp.tile([128, ntiles], F32)
            nc.sync.dma_start(bloc[:], bloc_d[:])

            psum_o = pop.tile([SPC, D + 1], F32)
            e_tiles = {}
            xT_tiles = {}
            th_tiles = {}

            def nat_dma(t):
                r0 = t * CHUNK
                xn = slots[t % NSLOT]
                nc.sync.dma_start(
                    xn[:, :, 0:D],
                    x_d[r0 : r0 + CHUNK, :].rearrange("(a p) d -> p a d", p=128),
                )

            def stage_tp(t):
                # PE transposes of the natural tiles -> PSUM, then DVE
                # copy/cast into the xT sbuf tile (feature halves contiguous)
                xn = slots[t % NSLOT]
                xT = xTp.tile([128, 2, CHUNK], BF16)
                xT_tiles[t] = xT
                for q in range(TPC // 4):  # four node tiles per 2-bank tile
                    tp = tpp.tile([128, 4, 2, 128], BF16)  # [p, jj, h, n]
                    for jj in range(4):
                        j = q * 4 + jj
                        nc.tensor.transpose(
                            tp[:, jj, 0, :], xn[:, j, 0:128], ident[:]
                        )
                        nc.tensor.transpose(
                            tp[:, jj, 1, :], xn[:, j, 128:256], ident[:]
                        )
                    # one copy per psum tile: out free dims (jj, h, n)
                    nc.vector.tensor_copy(
                        xT[:, :, q * 512 : (q + 1) * 512].rearrange(
                            "p h (jj n) -> p jj h n", jj=4
                        ),
                        tp[:],
                    )

            def stage_h(t):
                xT = xT_tiles[t]
                th = thp.tile([H, CHUNK], BF16)
                th_tiles[t] = th
                for u in range(CHUNK // 512):
                    ph = php.tile([H, 512], F32)
                    nc.tensor.matmul(
                        ph[:], w1a[:], xT[:, 0, u * 512 : (u + 1) * 512],
                        start=True, stop=False,
                    )
                    nc.tensor.matmul(
                        ph[:], w1b[:], xT[:, 1, u * 512 : (u + 1) * 512],
                        start=False, stop=True,
                    )
                    nc.scalar.activation(
                        th[:, u * 512 : (u + 1) * 512], ph[:], TANH,
                        bias=b1[:], scale=1.0,
                    )
                del xT_tiles[t]

            def stage_s(t):
                th = th_tiles.pop(t)
                ps = psp.tile([128, TPC], F32)
                for j in range(TPC):
                    nc.tensor.matmul(
                        ps[:, j : j + 1],
                        th[:, j * 128 : (j + 1) * 128],
                        w2[:],
                        start=True,
                        stop=True,
                    )
                eb = ebp.tile([128, TPC], F32)
                e_tiles[t] = eb
                nc.scalar.activation(eb[:], ps[:], EXP, bias=0.0, scale=1.0)

            def stage_sw(t):
                sws = []
                eb = e_tiles[t]
                for j in range(TPC):
                    g = t * TPC + j
                    sw = swp.tile([128, SPC], BF16)
                    nc.vector.tensor_scalar(
                        sw[:],
                        iota[:],
                        bloc[:, g : g + 1],
                        eb[:, j : j + 1],
                        AluOpType.is_equal,
                        AluOpType.mult,
                    )
                    sws.append(sw)
                del e_tiles[t]
                return sws

            def stage_o(t, sws):
                xn = slots[t % NSLOT]
                for j in range(TPC):
                    nc.tensor.matmul(
                        psum_o[:],
                        sws[j],
                        xn[:, j, :],
                        start=(t == 0 and j == 0),
                        stop=(t == nchunks - 1 and j == TPC - 1),
                    )

            LAG_H, LAG_S, LAG_O = 1, 2, 4
            for t in range(nchunks + LAG_O):
                if t + 2 < nchunks:
                    nat_dma(t + 2)
                k = t - LAG_O
                sws = stage_sw(k) if 0 <= k else None
                if t < nchunks:
                    stage_tp(t)
                if 0 <= t - LAG_H < nchunks:
                    stage_h(t - LAG_H)
                if 0 <= t - LAG_S < nchunks:
                    stage_s(t - LAG_S)
                if sws is not None:
                    stage_o(k, sws)

            dent = finp.tile([SPC, 1], F32)
            nc.vector.tensor_scalar(
                dent[:],
                psum_o[:, D : D + 1],
                cnt[:],
                1e-30,
                AluOpType.mult,
                AluOpType.max,
            )
            rec = finp.tile([SPC, 1], F32)
            nc.vector.reciprocal(rec[:], dent[:])
            osb = finp.tile([SPC, D], F32)
            nc.vector.tensor_scalar_mul(osb[:], psum_o[:, 0:D], rec[:])
            nc.sync.dma_start(o_d[:], osb[:])

    nc.compile()
    return nc


def kernel(x, batch, W1, b1, W2, b2):
    x = np.asarray(x)
    batch = np.asarray(batch)
    W1 = np.asarray(W1, np.float32)
    b1 = np.asarray(b1, np.float32)
    W2 = np.asarray(W2, np.float32)
    b2 = np.asarray(b2, np.float32)

    bat = batch.astype(np.int64)
    bounds = np.searchsorted(bat, np.arange(0, G + 1, SPC), side="left")
    ncounts = np.diff(bounds)
    npad = int(-(-ncounts.max() // CHUNK) * CHUNK)
    ntiles = npad // 128

    counts = np.bincount(bat, minlength=G).astype(np.float32)

    # b2 shifts every score equally; softmax is shift-invariant, so it is
    # mathematically irrelevant to the output and never sent to the device
    if npad not in _cache:
        _cache[npad] = _build(npad)
    nc = _cache[npad]

    x_bf = x.astype(ml_dtypes.bfloat16)
    w1a = W1[0:128, :].astype(ml_dtypes.bfloat16)
    w1b = W1[128:256, :].astype(ml_dtypes.bfloat16)
    w2 = W2.reshape(H, 1).astype(ml_dtypes.bfloat16)
    b1c = b1.reshape(H, 1).astype(np.float32)
    ident = np.eye(128, dtype=ml_dtypes.bfloat16)
    iota = np.broadcast_to(
        np.arange(SPC, dtype=np.float32)[None, :], (128, SPC)
    ).copy()

    in_maps = []
    for c in range(C):
        s, e = bounds[c], bounds[c + 1]
        nct = e - s
        xc = np.zeros((npad, D), ml_dtypes.bfloat16)
        xc[:nct] = x_bf[s:e]
        blc = np.full((npad,), -1.0, np.float32)
        blc[:nct] = (bat[s:e] - c * SPC).astype(np.float32)
        blc = np.ascontiguousarray(blc.reshape(ntiles, 128).T)
        cntc = np.maximum(counts[c * SPC : (c + 1) * SPC], 1.0).reshape(SPC, 1)
        in_maps.append(
            {
                "x": xc,
                "bloc": blc,
                "w1a": w1a,
                "w1b": w1b,
                "w2": w2,
                "b1": b1c,
                "ident": ident,
                "iota": iota,
                "cnt": cntc,
            }
        )

    res = bass_utils.run_bass_kernel_spmd(nc, in_maps, core_ids=list(range(C)))
    out = np.concatenate([res.results[c]["o"] for c in range(C)], axis=0)
    return out.astype(np.float32)

